# revision 7
# baseline (speedup 1.0000x reference)
"""Fused single-NEFF Bass/Trainium2 kernel for nn_ChannelAttention.

Math: per (batch b, 32-channel block n), q/k/v are per-channel affine maps of
x rows, so the whole module collapses to out[b] = M_b @ x[b] + beta_b with M_b
block-diagonal (4x 32x32), where M_b derives from the per-batch channel Gram
G = X X^T and row sums S = X @ 1 via 16 tiny softmaxes.

Single NEFF, sharded over pixels (8192 px/core):
  1. x loaded ONCE: HWDGE fp32 -> SBUF staging, engine-cast to fp16, kept
     resident (phase 2 reuses it; the baseline's second 8 MiB HBM read of x
     is gone).  PE transposes 128-px chunks (identity matmul) and accumulates
     the per-batch Gram [G|S] in PSUM (fp16 in, fp32 accum).
  2. Per-batch AllReduce of [128,129] Gram stats across the 8 cores (66 KB).
  3. M built ON DEVICE from the reduced G,S: row-broadcasts via tiny PE
     matmuls (transpose + one-hot selector), logits assembled with fused
     scalar_tensor_tensor ops, exp+rowsum on ACT (off-block logits are exactly
     0 by construction -> rowsum correction is exactly -96), P folded with
     host-precomputed w_fus*V / w_fus*U masks.  All cores compute identical M.
  4. out = M^T.T @ x + beta from SBUF-resident fp16 x, fp32 out to HBM.
"""

import numpy as np

import concourse.bacc as bacc
import concourse.mybir as mybir
import concourse.tile as tile
import concourse.bass_utils as bass_utils

B, C, H, W = 2, 128, 256, 256
HW = H * W
NCORES = 8
SH = HW // NCORES      # 8192 pixels per core
E = 2
NCH = 4
D = C // NCH           # 32
NFULL = float(HW)
F32 = mybir.dt.float32
FP16 = mybir.dt.float16

CH = 2048              # dma chunk columns
GRP = 4                # 128-px chunks per transpose group
AL = mybir.AluOpType

# ccols column indices
CCOL = {}
_i = 0
for _e in range(E):
    for _nm in (f"c{_e}", f"nd{_e}", f"ta{_e}", f"tb{_e}",
                f"a2{_e}", f"ab2{_e}", f"b2n{_e}",
                f"c2{_e}", f"cd2{_e}", f"d2n{_e}"):
        CCOL[_nm] = _i
        _i += 1
CCOL["bfus"] = _i; _i += 1
CCOL["bi"] = _i; _i += 4   # BI4 block indicators, 4 cols
NCC = _i

CMAT = {"crow0": 0, "drow0": 1, "wv0": 2, "uw0": 3,
        "crow1": 4, "drow1": 5, "wv1": 6, "uw1": 7, "ident": 8}
NCM = 9

_cache = {}


def _build_fused():
    nc = bacc.Bacc("TRN2", target_bir_lowering=False, debug=False,
                   num_devices=NCORES)
    x = nc.dram_tensor("x", [B, C, SH], F32, kind="ExternalInput").ap()
    idd = nc.dram_tensor("idd", [C, C], FP16, kind="ExternalInput").ap()
    cmats = nc.dram_tensor("cmats", [C, NCM, C], F32, kind="ExternalInput").ap()
    ccols = nc.dram_tensor("ccols", [C, NCC], F32, kind="ExternalInput").ap()
    selm = nc.dram_tensor("selm", [10, 4 * C], FP16, kind="ExternalInput").ap()
    out = nc.dram_tensor("out", [B, C, SH], F32, kind="ExternalOutput").ap()

    with tile.TileContext(nc) as tc:
        with (
            tc.tile_pool(name="const", bufs=1) as constp,
            tc.tile_pool(name="xres", bufs=1) as xresp,
            tc.tile_pool(name="xstage", bufs=3) as xstagep,
            tc.tile_pool(name="xt", bufs=3) as xtp,
            tc.tile_pool(name="gsb", bufs=1) as gsbp,
            tc.tile_pool(name="work", bufs=4) as workp,
            tc.tile_pool(name="small", bufs=8) as smallp,
            tc.tile_pool(name="persist", bufs=1) as perp,
            tc.tile_pool(name="osb", bufs=3) as osbp,
            tc.tile_pool(name="xtps", bufs=2, space="PSUM") as xtpsp,
            tc.tile_pool(name="gram", bufs=1, space="PSUM") as gramp,
            tc.tile_pool(name="bc", bufs=2, space="PSUM") as bcp,
            tc.tile_pool(name="aux", bufs=1, space="PSUM") as auxp,
            tc.tile_pool(name="p2", bufs=2, space="PSUM") as p2p,
            tc.tile_pool(name="dram", bufs=1, space="DRAM") as dramp,
        ):
            ident = constp.tile([C, C], FP16, tag="ident")
            nc.sync.dma_start(out=ident, in_=idd)
            cm = constp.tile([C, NCM, C], F32, tag="cm")
            nc.sync.dma_start(out=cm, in_=cmats)
            cc = constp.tile([C, NCC], F32, tag="cc")
            nc.sync.dma_start(out=cc, in_=ccols)
            sel = constp.tile([10, 4 * C], FP16, tag="sel")
            nc.sync.dma_start(out=sel, in_=selm)

            def cmx(name):
                return cm[:, CMAT[name], :]

            def ccx(name, w=1):
                j = CCOL[name]
                return cc[:, j:j + w]

            xres = [[None] * (SH // CH) for _ in range(B)]
            state = {"gabs": 0, "cast_tog": 0, "st_tog": 0, "cp_rot": 0}

            # ---------------- phase 1: load + cast + transpose + gram ------
            def emit_load_cast(b, jc):
                xst = xstagep.tile([C, CH], F32, tag="xst")
                if b == 0 and jc == 0:
                    splits = (512, 512, 1024)
                else:
                    splits = (CH,)
                w0 = 0
                for wd in splits:
                    nc.sync.dma_start(out=xst[:, w0:w0 + wd],
                                      in_=x[b, :, jc * CH + w0:jc * CH + w0 + wd])
                    w0 += wd
                xr = xresp.tile([C, CH], FP16, tag=f"xr{b}_{jc}")
                for s in range(4):
                    c0 = s * (CH // 4)
                    dst = xr[:, c0:c0 + CH // 4]
                    src = xst[:, c0:c0 + CH // 4]
                    if state["cast_tog"] % 2 == 0:
                        nc.scalar.copy(dst, src)
                    else:
                        nc.gpsimd.tensor_copy(dst, src)
                    state["cast_tog"] += 1
                xres[b][jc] = xr

            def emit_grams(args, b):
                gram_t, xt_sb, j0 = args
                for i in range(GRP):
                    j = j0 + i
                    nc.tensor.matmul(gram_t[:, 0:129],
                                     lhsT=xt_sb[:, i, 0:128],
                                     rhs=xt_sb[:, i, 0:129],
                                     start=(j == 0), stop=(j == SH // 128 - 1))

            def emit_tg_groups(b, jcs, pend):
                # pend: [gram_tile, pending_args]
                for jc in jcs:
                    emit_load_cast(b, jc)
                    for kg in range(CH // 128 // GRP):   # 4 groups per chunk
                        xt_ps = xtpsp.tile([C, 512], F32, tag="xtps")
                        for i in range(GRP):
                            k = kg * GRP + i
                            nc.tensor.matmul(
                                xt_ps[:, i * 128:(i + 1) * 128],
                                lhsT=xres[b][jc][:, k * 128:(k + 1) * 128],
                                rhs=ident, start=True, stop=True)
                        if pend[1] is not None:
                            emit_grams(pend[1], b)
                        xt_sb = xtp.tile([C, GRP, 132], FP16, tag="xt")
                        nc.vector.tensor_copy(
                            xt_sb[:, :, 0:128],
                            xt_ps.rearrange("p (g f) -> p g f", g=GRP))
                        if state["gabs"] < 3:
                            nc.vector.memset(xt_sb[:, :, 128:129], 1.0)
                        state["gabs"] += 1
                        g = jc * (CH // 128 // GRP) + kg
                        pend[1] = (pend[0], xt_sb, g * GRP)

            # ---------------- allreduce --------------------------------
            gsb = [None] * B

            def emit_allreduce(b, gram_t):
                gs_sb = gsbp.tile([C, 129], F32, tag=f"gs{b}")
                nc.vector.tensor_copy(gs_sb, gram_t[:, 0:129])
                gin = dramp.tile([C, 129], F32, tag=f"gin{b}")
                gout = dramp.tile([C, 129], F32, tag=f"gout{b}")
                nc.gpsimd.dma_start(out=gin, in_=gs_sb)
                nc.gpsimd.collective_compute(
                    "AllReduce", AL.add,
                    replica_groups=[list(range(NCORES))],
                    ins=[gin.opt()], outs=[gout.opt()])
                g_sb = gsbp.tile([C, 129], F32, tag=f"G{b}")
                nc.gpsimd.dma_start(out=g_sb, in_=gout)
                gsb[b] = g_sb

            # ---------------- M build ----------------------------------
            mb = [dict() for _ in range(B)]

            def emit_mbuild_pre(b):
                d = mb[b]
                G = gsb[b][:, 0:128]
                S = gsb[b][:, 128:129]
                junk = workp.tile([C, C], F32, tag="w")
                dG = perp.tile([C, 1], F32, tag="dG")
                nc.vector.scalar_tensor_tensor(
                    out=junk, in0=G, scalar=1.0, in1=cmx("ident"),
                    op0=AL.mult, op1=AL.mult, accum_out=dG)
                stack = perp.tile([C, 10], FP16, tag="stack")
                for e in range(E):
                    # nq2 = a2*dG + (ab2*S + b2n) ; rnq = 1/sqrt(nq2)
                    t_a = smallp.tile([C, 1], F32, tag="sm")
                    nc.vector.scalar_tensor_tensor(
                        out=t_a, in0=ccx(f"ab2{e}"), scalar=S,
                        in1=ccx(f"b2n{e}"), op0=AL.mult, op1=AL.add)
                    nq2 = smallp.tile([C, 1], F32, tag="sm")
                    nc.vector.scalar_tensor_tensor(
                        out=nq2, in0=ccx(f"a2{e}"), scalar=dG, in1=t_a,
                        op0=AL.mult, op1=AL.add)
                    inq2 = smallp.tile([C, 1], F32, tag="sm")
                    nc.vector.reciprocal(inq2, nq2)
                    rnq = smallp.tile([C, 1], F32, tag="sm")
                    nc.scalar.sqrt(rnq, inq2)
                    t_c = smallp.tile([C, 1], F32, tag="sm")
                    nc.vector.scalar_tensor_tensor(
                        out=t_c, in0=ccx(f"cd2{e}"), scalar=S,
                        in1=ccx(f"d2n{e}"), op0=AL.mult, op1=AL.add)
                    nk2 = smallp.tile([C, 1], F32, tag="sm")
                    nc.vector.scalar_tensor_tensor(
                        out=nk2, in0=ccx(f"c2{e}"), scalar=dG, in1=t_c,
                        op0=AL.mult, op1=AL.add)
                    ink2 = smallp.tile([C, 1], F32, tag="sm")
                    nc.vector.reciprocal(ink2, nk2)
                    rnk = smallp.tile([C, 1], F32, tag="sm")
                    nc.scalar.sqrt(rnk, ink2)
                    aq = perp.tile([C, 1], F32, tag=f"aq{e}")
                    nc.vector.tensor_scalar(out=aq, in0=ccx(f"ta{e}"),
                                            scalar1=rnq, scalar2=None,
                                            op0=AL.mult)
                    bq = perp.tile([C, 1], F32, tag=f"bq{e}")
                    nc.vector.tensor_scalar(out=bq, in0=ccx(f"tb{e}"),
                                            scalar1=rnq, scalar2=None,
                                            op0=AL.mult)
                    d[f"aq{e}"], d[f"bq{e}"] = aq, bq
                    # stack cols: cs_e at col e ; rnk_e * BI4 at cols 2+4e..
                    nc.vector.scalar_tensor_tensor(
                        out=stack[:, e:e + 1], in0=ccx(f"c{e}"), scalar=S,
                        in1=ccx(f"nd{e}"), op0=AL.mult, op1=AL.add)
                    nc.vector.tensor_scalar(
                        out=stack[:, 2 + 4 * e:6 + 4 * e], in0=ccx("bi", 4),
                        scalar1=rnk, scalar2=None, op0=AL.mult)
                d["stack"] = stack
                d["S"] = S
                d["G"] = G

            def emit_bcast(b):
                d = mb[b]
                aux = auxp.tile([C, 512], F32, tag="aux")
                nc.tensor.matmul(aux[0:10, 0:128], lhsT=d["stack"], rhs=ident,
                                 start=True, stop=True)
                rt = perp.tile([10, C], FP16, tag="rt")
                nc.vector.tensor_copy(rt, aux[0:10, 0:128])
                for e in range(E):
                    bc = bcp.tile([C, 512], F32, tag="bc")
                    nc.tensor.matmul(bc[:, 0:128],
                                     lhsT=sel[:, 256 * e:256 * e + 128],
                                     rhs=rt, start=True, stop=True)
                    nc.tensor.matmul(bc[:, 128:256],
                                     lhsT=sel[:, 256 * e + 128:256 * e + 256],
                                     rhs=rt, start=True, stop=True)
                    d[f"bc{e}"] = bc

            def emit_mchain(b):
                d = mb[b]
                G, S = d["G"], d["S"]
                maccs = []
                for e in range(E):
                    csrow = d[f"bc{e}"][:, 0:128]
                    rnkrow = d[f"bc{e}"][:, 128:256]
                    w0 = workp.tile([C, C], F32, tag="w")
                    nc.vector.tensor_tensor(out=w0, in0=G, in1=cmx(f"crow{e}"),
                                            op=AL.mult)
                    w1 = workp.tile([C, C], F32, tag="w")
                    nc.vector.scalar_tensor_tensor(
                        out=w1, in0=cmx(f"drow{e}"), scalar=S, in1=w0,
                        op0=AL.mult, op1=AL.add)
                    w2 = workp.tile([C, C], F32, tag="w")
                    nc.vector.tensor_scalar(out=w2, in0=csrow,
                                            scalar1=d[f"bq{e}"], scalar2=None,
                                            op0=AL.mult)
                    w3 = workp.tile([C, C], F32, tag="w")
                    nc.vector.scalar_tensor_tensor(
                        out=w3, in0=w1, scalar=d[f"aq{e}"], in1=w2,
                        op0=AL.mult, op1=AL.add)
                    w4 = workp.tile([C, C], F32, tag="w")
                    nc.vector.tensor_tensor(out=w4, in0=w3, in1=rnkrow,
                                            op=AL.mult)
                    w5 = workp.tile([C, C], F32, tag="w")
                    rsraw = smallp.tile([C, 1], F32, tag="sm")
                    nc.scalar.activation(out=w5, in_=w4,
                                         func=mybir.ActivationFunctionType.Exp,
                                         accum_out=rsraw)
                    rs = smallp.tile([C, 1], F32, tag="sm")
                    nc.vector.tensor_scalar_add(rs, in0=rsraw, scalar1=-96.0)
                    rp = smallp.tile([C, 1], F32, tag="sm")
                    nc.vector.reciprocal(rp, rs)
                    macc = perp.tile([C, C], F32, tag=f"macc{e}")
                    nc.vector.scalar_tensor_tensor(
                        out=macc, in0=w5, scalar=rp, in1=cmx(f"wv{e}"),
                        op0=AL.mult, op1=AL.mult)
                    maccs.append(macc)
                    w7 = workp.tile([C, C], F32, tag="w")
                    bacc = perp.tile([C, 1], F32, tag=f"bacc{e}")
                    nc.vector.scalar_tensor_tensor(
                        out=w7, in0=w5, scalar=rp, in1=cmx(f"uw{e}"),
                        op0=AL.mult, op1=AL.mult, accum_out=bacc)
                    d[f"bacc{e}"] = bacc
                mcast = perp.tile([C, C], FP16, tag="mcast")
                nc.vector.tensor_tensor(out=mcast, in0=maccs[0], in1=maccs[1],
                                        op=AL.add)
                d["mcast"] = mcast
                tmpb = smallp.tile([C, 1], F32, tag="sm")
                nc.vector.tensor_tensor(out=tmpb, in0=d["bacc0"],
                                        in1=d["bacc1"], op=AL.add)
                beta = perp.tile([C, 1], F32, tag=f"beta{b}")
                nc.vector.tensor_tensor(out=beta, in0=tmpb, in1=ccx("bfus"),
                                        op=AL.add)
                d["beta"] = beta

            def emit_mt(b):
                d = mb[b]
                aux = auxp.tile([C, 512], F32, tag="aux")
                nc.tensor.matmul(aux[:, 0:128], lhsT=d["mcast"], rhs=ident,
                                 start=True, stop=True)
                mt_sb = perp.tile([C, C], FP16, tag=f"mt{b}")
                nc.vector.tensor_copy(mt_sb, aux[:, 0:128])
                d["mt"] = mt_sb

            # ---------------- phase 2 ----------------------------------
            def emit_p2(b, jcs):
                d = mb[b]
                for jc in jcs:
                    o_sb = osbp.tile([C, CH], F32, tag="osb")
                    for k in range(CH // 512):
                        ps = p2p.tile([C, 512], F32, tag="ps")
                        nc.tensor.matmul(
                            ps, lhsT=d["mt"],
                            rhs=xres[b][jc][:, k * 512:(k + 1) * 512],
                            start=True, stop=True)
                        dst = o_sb[:, k * 512:(k + 1) * 512]
                        r = state["cp_rot"] % 2
                        state["cp_rot"] += 1
                        if r == 0:
                            nc.vector.tensor_scalar_add(dst, in0=ps,
                                                        scalar1=d["beta"])
                        else:
                            nc.scalar.add(dst, ps, d["beta"])
                    dst_d = out[b, :, jc * CH:(jc + 1) * CH]
                    if state["st_tog"] % 2 == 0:
                        nc.scalar.dma_start(out=dst_d, in_=o_sb)
                    else:
                        nc.sync.dma_start(out=dst_d, in_=o_sb)
                    state["st_tog"] += 1

            # ================= emission schedule =======================
            gram0 = gramp.tile([C, 512], F32, tag="gram")
            pend0 = [gram0, None]
            emit_tg_groups(0, [0, 1, 2, 3], pend0)
            emit_grams(pend0[1], 0)
            emit_allreduce(0, gram0)

            gram1 = gramp.tile([C, 512], F32, tag="gram")
            pend1 = [gram1, None]
            emit_tg_groups(1, [0, 1], pend1)

            emit_mbuild_pre(0)
            emit_bcast(0)
            emit_mchain(0)

            emit_tg_groups(1, [2, 3], pend1)
            emit_grams(pend1[1], 1)
            emit_allreduce(1, gram1)

            emit_mt(0)
            emit_p2(0, [0, 1])

            emit_mbuild_pre(1)
            emit_bcast(1)

            emit_p2(0, [2, 3])

            emit_mchain(1)
            emit_mt(1)
            emit_p2(1, [0, 1, 2, 3])

    nc.compile()
    return nc


def _host_consts(w_qkv, b_qkv, w_fus, b_fus, t):
    """Pack host-side constants. All math in float64."""
    t = t.reshape(E * NCH)
    blk = np.arange(C) // D                      # block index per channel
    BM = (blk[:, None] == blk[None, :]).astype(np.float64)

    cmats = np.zeros((C, NCM, C), np.float64)
    ccols = np.zeros((C, NCC), np.float64)
    selm = np.zeros((10, 4 * C), np.float64)
    cmats[:, CMAT["ident"], :] = np.eye(C)
    ccols[:, CCOL["bfus"]] = b_fus
    for n in range(NCH):
        ccols[:, CCOL["bi"] + n] = (blk == n).astype(np.float64)

    for e in range(E):
        A = w_qkv[:, e]; Bv = b_qkv[:, e]
        Cv = w_qkv[:, E + e]; Dv = b_qkv[:, E + e]
        Vv = w_qkv[:, 2 * E + e]; Uv = b_qkv[:, 2 * E + e]
        wf = w_fus[:, e]
        tau = t[e * NCH + blk]                   # per-channel temperature
        cmats[:, CMAT[f"crow{e}"], :] = BM * Cv[None, :]
        cmats[:, CMAT[f"drow{e}"], :] = BM * Dv[None, :]
        cmats[:, CMAT[f"wv{e}"], :] = BM * (wf[:, None] * Vv[None, :])
        cmats[:, CMAT[f"uw{e}"], :] = BM * (wf[:, None] * Uv[None, :])
        ccols[:, CCOL[f"c{e}"]] = Cv
        ccols[:, CCOL[f"nd{e}"]] = NFULL * Dv
        ccols[:, CCOL[f"ta{e}"]] = tau * A
        ccols[:, CCOL[f"tb{e}"]] = tau * Bv
        ccols[:, CCOL[f"a2{e}"]] = A * A
        ccols[:, CCOL[f"ab2{e}"]] = 2 * A * Bv
        ccols[:, CCOL[f"b2n{e}"]] = NFULL * Bv * Bv
        ccols[:, CCOL[f"c2{e}"]] = Cv * Cv
        ccols[:, CCOL[f"cd2{e}"]] = 2 * Cv * Dv
        ccols[:, CCOL[f"d2n{e}"]] = NFULL * Dv * Dv
        # selector matrices: stack rows are [cs0, cs1, rnk0*bi(4), rnk1*bi(4)]
        selm[e, 256 * e:256 * e + 128] = 1.0                  # sel_cs_e
        for cch in range(C):
            selm[2 + 4 * e + blk[cch], 256 * e + 128 + cch] = 1.0  # sel_rnk_e

    return (cmats.astype(np.float32), ccols.astype(np.float32),
            selm.astype(np.float16))


def kernel(x, w_qkv, b_qkv, w_fus, b_fus, t, _profile=None):
    x = np.asarray(x, dtype=np.float32)
    w_qkv = np.asarray(w_qkv, dtype=np.float64)
    b_qkv = np.asarray(b_qkv, dtype=np.float64)
    w_fus = np.asarray(w_fus, dtype=np.float64)
    b_fus = np.asarray(b_fus, dtype=np.float64)
    t = np.asarray(t, dtype=np.float64)

    if "fused" not in _cache:
        _cache["fused"] = _build_fused()
    nc = _cache["fused"]

    cmats, ccols, selm = _host_consts(w_qkv, b_qkv, w_fus, b_fus, t)
    idd = np.eye(C, dtype=np.float16)

    xf = x.reshape(B, C, HW)
    shards = [np.ascontiguousarray(xf[:, :, i * SH:(i + 1) * SH])
              for i in range(NCORES)]

    kw = {}
    if _profile and _profile.get("trace"):
        kw["trace"] = True
    res = bass_utils.run_bass_kernel_spmd(
        nc,
        [{"x": s, "idd": idd, "cmats": cmats, "ccols": ccols, "selm": selm}
         for s in shards],
        core_ids=list(range(NCORES)), **kw)
    out = np.concatenate([r["out"] for r in res.results], axis=2)
    if _profile is not None:
        _profile["results"] = res
    return out.reshape(B, C, H, W)


# revision 22
# speedup vs baseline: 1.7602x; 1.7602x over previous
"""Head-sharded single-NEFF Bass/Trainium2 kernel for nn_ChannelAttention.

Math: per (batch b, 32-channel block n), q/k/v are per-channel affine maps of
x rows, so the module collapses to out[b,blk] = M @ x[b,blk] + beta with M a
32x32 matrix derived from the block's channel Gram G = X X^T and row sums
S = X @ 1 via 2 tiny softmaxes (e = 0,1).

Sharding: B*NCH = 2*4 = 8 = exactly one (batch, block) per core -> each core
is fully independent (NO collectives, no cross-core rendezvous, which costs
~100us on this 8-core axon setup).  Per core: 8 MiB in + 8 MiB out, the
bidirectional-HBM roofline.

Host pre-stripes each core's x as [4 stripes x 32 ch = 128 partitions, 16384]
so all PE work runs 128 wide:
  - Gram: PE-transpose 128-px chunks, accumulate stripe-block Gram [128,129]
    in PSUM; fold the 4 stripes with one tiny matmul (lhsT = stripe-fold
    selector) -> [G|S] [32,33].
  - M build on 32-partition tiles: row-broadcasts via transpose+one-hot PE
    matmuls, logits via fused scalar_tensor_tensor, exp+rowsum on ACT.
  - Phase 2: M^T replicated into a block-diagonal [128,128] lhsT ->
    full-width matmuls against the resident fp16 x; fp32 out.
x is loaded ONCE (HWDGE fp32 + engine cast to resident fp16); phase 2 reads
it from SBUF, so HBM traffic is the 16 MiB/core minimum.
"""

import numpy as np

import concourse.bacc as bacc
import concourse.mybir as mybir
import concourse.tile as tile
import concourse.bass_utils as bass_utils

B, C, H, W = 2, 128, 256, 256
HW = H * W
NCORES = 8
E = 2
NCH = 4
D = C // NCH            # 32 channels per block
NST = 4                 # stripes per core
SW = HW // NST          # 16384 stripe width
NFULL = float(HW)
F32 = mybir.dt.float32
FP16 = mybir.dt.float16

CH = 2048               # dma chunk columns
GRP = 4                 # 128-px chunks per transpose group
NCHUNK = SW // 128      # 128 gram chunks
AL = mybir.AluOpType

# ccols column indices (per-core [32, NCC] consts)
CCOL = {}
_i = 0
for _e in range(E):
    for _nm in (f"c{_e}", f"nd{_e}", f"ta{_e}", f"tb{_e}",
                f"a2{_e}", f"ab2{_e}", f"b2n{_e}",
                f"c2{_e}", f"cd2{_e}", f"d2n{_e}"):
        CCOL[_nm] = _i
        _i += 1
CCOL["bfus"] = _i; _i += 1
NCC = _i

CMAT = {"crow0": 0, "drow0": 1, "wv0": 2, "uw0": 3,
        "crow1": 4, "drow1": 5, "wv1": 6, "uw1": 7, "ident": 8}
NCM = 9

_cache = {}


def _build():
    nc = bacc.Bacc("TRN2", target_bir_lowering=False, debug=False,
                   num_devices=NCORES)
    x = nc.dram_tensor("x", [C, SW], F32, kind="ExternalInput").ap()
    idd = nc.dram_tensor("idd", [C, C], FP16, kind="ExternalInput").ap()
    st4 = nc.dram_tensor("st4", [C, C], F32, kind="ExternalInput").ap()
    cmats = nc.dram_tensor("cmats", [C, NCM, D], F32, kind="ExternalInput").ap()
    ccols = nc.dram_tensor("ccols", [C, NCC], F32, kind="ExternalInput").ap()
    selm = nc.dram_tensor("selm", [NST, 4 * C], FP16, kind="ExternalInput").ap()
    out = nc.dram_tensor("out", [C, SW], F32, kind="ExternalOutput").ap()

    with tile.TileContext(nc) as tc:
        with (
            tc.tile_pool(name="const", bufs=1) as constp,
            tc.tile_pool(name="xres", bufs=1) as xresp,
            tc.tile_pool(name="xstage", bufs=3) as xstagep,
            tc.tile_pool(name="xt", bufs=3) as xtp,
            tc.tile_pool(name="work", bufs=4) as workp,
            tc.tile_pool(name="small", bufs=8) as smallp,
            tc.tile_pool(name="persist", bufs=1) as perp,
            tc.tile_pool(name="osb", bufs=3) as osbp,
            tc.tile_pool(name="xtps", bufs=2, space="PSUM") as xtpsp,
            tc.tile_pool(name="gram", bufs=1, space="PSUM") as gramp,
            tc.tile_pool(name="aux", bufs=2, space="PSUM") as auxp,
            tc.tile_pool(name="p2", bufs=3, space="PSUM") as p2p,
        ):
            ident = constp.tile([C, C], FP16, tag="ident")
            nc.sync.dma_start(out=ident, in_=idd)
            stack4 = constp.tile([C, C], F32, tag="st4")
            nc.sync.dma_start(out=stack4, in_=st4)
            cm = constp.tile([C, NCM, D], F32, tag="cm")
            nc.sync.dma_start(out=cm, in_=cmats)
            cc = constp.tile([C, NCC], F32, tag="cc")
            nc.sync.dma_start(out=cc, in_=ccols)
            sel = constp.tile([NST, 4 * C], FP16, tag="sel")
            nc.sync.dma_start(out=sel, in_=selm)

            def cmx(name):
                return cm[:, CMAT[name], :]

            def ccx(name, w=1):
                j = CCOL[name]
                return cc[:, j:j + w]

            # M4: block-diagonal phase-2 weights; zero the off-blocks once.
            m4 = perp.tile([C, C], FP16, tag="m4")
            nc.vector.memset(m4, 0.0)
            # warm the ACT Sqrt table early so the mid-kernel sqrt is cheap
            wrm = smallp.tile([D, 1], F32, tag="sm")
            nc.vector.memset(wrm, 1.0)
            wrm2 = smallp.tile([D, 1], F32, tag="sm")
            nc.scalar.sqrt(wrm2, wrm)

            xres = [None] * (SW // CH)
            state = {"cast_tog": 0, "st_tog": 0, "cp_rot": 0, "xt_n": 0}

            # ---------------- phase 1: load + cast + transpose + gram ------
            def emit_load_cast(jc):
                xst = xstagep.tile([C, CH], F32, tag="xst")
                if jc == 0:
                    splits = (512, 512, 1024)
                else:
                    splits = (CH,)
                w0 = 0
                for wd in splits:
                    nc.sync.dma_start(out=xst[:, w0:w0 + wd],
                                      in_=x[:, jc * CH + w0:jc * CH + w0 + wd])
                    w0 += wd
                xr = xresp.tile([C, CH], FP16, tag=f"xr{jc}")
                for s in range(4):
                    c0 = s * (CH // 4)
                    dst = xr[:, c0:c0 + CH // 4]
                    src = xst[:, c0:c0 + CH // 4]
                    if state["cast_tog"] % 2 == 0:
                        nc.scalar.copy(dst, src)
                    else:
                        nc.gpsimd.tensor_copy(dst, src)
                    state["cast_tog"] += 1
                xres[jc] = xr

            gram = gramp.tile([C, 512], F32, tag="gram")

            def emit_grams(args):
                xt_sb, j0 = args
                for i in range(GRP):
                    j = j0 + i
                    nc.tensor.matmul(gram[:, 0:129],
                                     lhsT=xt_sb[:, i, 0:128],
                                     rhs=xt_sb[:, i, 0:129],
                                     start=(j == 0), stop=(j == NCHUNK - 1))

            pend = [None]

            def emit_tg_groups(jcs):
                for jc in jcs:
                    emit_load_cast(jc)
                    for kg in range(CH // 128 // GRP):   # 4 groups per chunk
                        xt_ps = xtpsp.tile([C, 512], F32, tag="xtps")
                        for i in range(GRP):
                            k = kg * GRP + i
                            nc.tensor.matmul(
                                xt_ps[:, i * 128:(i + 1) * 128],
                                lhsT=xres[jc][:, k * 128:(k + 1) * 128],
                                rhs=ident, start=True, stop=True)
                        if pend[0] is not None:
                            emit_grams(pend[0])
                        xt_sb = xtp.tile([C, GRP, 132], FP16, tag="xt")
                        nc.vector.tensor_copy(
                            xt_sb[:, :, 0:128],
                            xt_ps.rearrange("p (g f) -> p g f", g=GRP))
                        if state["xt_n"] < 3:
                            nc.vector.memset(xt_sb[:, :, 128:129], 1.0)
                        state["xt_n"] += 1
                        g = jc * (CH // 128 // GRP) + kg
                        pend[0] = (xt_sb, g * GRP)

            # ---------------- gram fold + M build -------------------------
            d = {}

            def emit_fold():
                # extract the 4 stripe-diagonal [32,32] blocks (+ S col)
                gp = perp.tile([C, 33], F32, tag="gp")
                for s in range(NST):
                    nc.vector.tensor_copy(
                        gp[s * D:(s + 1) * D, 0:32],
                        gram[s * D:(s + 1) * D, s * D:s * D + 32])
                    nc.vector.tensor_copy(
                        gp[s * D:(s + 1) * D, 32:33],
                        gram[s * D:(s + 1) * D, 128:129])
                # fold stripes AND replicate to all 4 stripe positions:
                # out[32s'+j, i] = sum_s gp[32s+j, i]
                aux = auxp.tile([C, 512], F32, tag="aux")
                nc.tensor.matmul(aux[:, 0:33], lhsT=stack4, rhs=gp,
                                 start=True, stop=True)
                gs = perp.tile([C, 33], F32, tag="gs")
                nc.vector.tensor_copy(gs, aux[:, 0:33])
                d["G"] = gs[:, 0:32]
                d["S"] = gs[:, 32:33]

            def emit_mbuild():
                # everything [128, *]: values replicated across the 4 stripes
                G, S = d["G"], d["S"]
                junk = workp.tile([C, D], F32, tag="w")
                dG = perp.tile([C, 1], F32, tag="dG")
                nc.vector.scalar_tensor_tensor(
                    out=junk, in0=G, scalar=1.0, in1=cmx("ident"),
                    op0=AL.mult, op1=AL.mult, accum_out=dG)
                # nq2/nk2 for both e packed as [128,4] -> one recip + one sqrt
                n2 = perp.tile([C, 4], F32, tag="n2")
                for e in range(E):
                    t_a = smallp.tile([C, 1], F32, tag="sm")
                    nc.vector.scalar_tensor_tensor(
                        out=t_a, in0=ccx(f"ab2{e}"), scalar=S,
                        in1=ccx(f"b2n{e}"), op0=AL.mult, op1=AL.add)
                    nc.vector.scalar_tensor_tensor(
                        out=n2[:, 2 * e:2 * e + 1], in0=ccx(f"a2{e}"),
                        scalar=dG, in1=t_a, op0=AL.mult, op1=AL.add)
                    t_c = smallp.tile([C, 1], F32, tag="sm")
                    nc.vector.scalar_tensor_tensor(
                        out=t_c, in0=ccx(f"cd2{e}"), scalar=S,
                        in1=ccx(f"d2n{e}"), op0=AL.mult, op1=AL.add)
                    nc.vector.scalar_tensor_tensor(
                        out=n2[:, 2 * e + 1:2 * e + 2], in0=ccx(f"c2{e}"),
                        scalar=dG, in1=t_c, op0=AL.mult, op1=AL.add)
                in2 = perp.tile([C, 4], F32, tag="in2")
                nc.vector.reciprocal(in2, n2)
                rn = perp.tile([C, 4], F32, tag="rn")
                nc.scalar.sqrt(rn, in2)      # [rnq0, rnk0, rnq1, rnk1]
                stack = perp.tile([C, 4], FP16, tag="stack")
                for e in range(E):
                    rnq = rn[:, 2 * e:2 * e + 1]
                    rnk = rn[:, 2 * e + 1:2 * e + 2]
                    aq = perp.tile([C, 1], F32, tag=f"aq{e}")
                    nc.vector.tensor_scalar(out=aq, in0=ccx(f"ta{e}"),
                                            scalar1=rnq, scalar2=None,
                                            op0=AL.mult)
                    bq = perp.tile([C, 1], F32, tag=f"bq{e}")
                    nc.vector.tensor_scalar(out=bq, in0=ccx(f"tb{e}"),
                                            scalar1=rnq, scalar2=None,
                                            op0=AL.mult)
                    d[f"aq{e}"], d[f"bq{e}"] = aq, bq
                    # stack cols: cs_e at col e ; rnk_e at col 2+e
                    nc.vector.scalar_tensor_tensor(
                        out=stack[:, e:e + 1], in0=ccx(f"c{e}"), scalar=S,
                        in1=ccx(f"nd{e}"), op0=AL.mult, op1=AL.add)
                    nc.vector.tensor_scalar(
                        out=stack[:, 2 + e:3 + e], in0=rnk, scalar1=1.0,
                        scalar2=None, op0=AL.mult)
                # transpose stack -> RT [4,128]; row-broadcast via one-hots
                # (RT cols 0:32 = stripe-0 values, same as any stripe)
                aux = auxp.tile([C, 512], F32, tag="aux")
                nc.tensor.matmul(aux[0:4, 0:128], lhsT=stack,
                                 rhs=ident, start=True, stop=True)
                rt = perp.tile([NST, D], FP16, tag="rt")
                nc.vector.tensor_copy(rt, aux[0:4, 0:32])
                bc = auxp.tile([C, 512], F32, tag="aux")
                for e in range(E):
                    nc.tensor.matmul(bc[:, 128 * e:128 * e + 32],
                                     lhsT=sel[:, 256 * e:256 * e + 128],
                                     rhs=rt, start=True, stop=True)
                    nc.tensor.matmul(bc[:, 128 * e + 32:128 * e + 64],
                                     lhsT=sel[:, 256 * e + 128:256 * e + 256],
                                     rhs=rt, start=True, stop=True)
                # chain per e
                maccs = []
                for e in range(E):
                    csrow = bc[:, 128 * e:128 * e + 32]
                    rnkrow = bc[:, 128 * e + 32:128 * e + 64]
                    w0 = workp.tile([C, D], F32, tag="w")
                    nc.vector.tensor_tensor(out=w0, in0=G, in1=cmx(f"crow{e}"),
                                            op=AL.mult)
                    w1 = workp.tile([C, D], F32, tag="w")
                    nc.vector.scalar_tensor_tensor(
                        out=w1, in0=cmx(f"drow{e}"), scalar=S, in1=w0,
                        op0=AL.mult, op1=AL.add)
                    w2 = workp.tile([C, D], F32, tag="w")
                    nc.vector.tensor_scalar(out=w2, in0=csrow,
                                            scalar1=d[f"bq{e}"], scalar2=None,
                                            op0=AL.mult)
                    w3 = workp.tile([C, D], F32, tag="w")
                    nc.vector.scalar_tensor_tensor(
                        out=w3, in0=w1, scalar=d[f"aq{e}"], in1=w2,
                        op0=AL.mult, op1=AL.add)
                    w4 = workp.tile([C, D], F32, tag="w")
                    nc.vector.tensor_tensor(out=w4, in0=w3, in1=rnkrow,
                                            op=AL.mult)
                    w5 = workp.tile([C, D], F32, tag="w")
                    rs = smallp.tile([C, 1], F32, tag="sm")
                    nc.scalar.activation(out=w5, in_=w4,
                                         func=mybir.ActivationFunctionType.Exp,
                                         accum_out=rs)
                    rp = smallp.tile([C, 1], F32, tag="sm")
                    nc.vector.reciprocal(rp, rs)
                    macc = perp.tile([C, D], F32, tag=f"macc{e}")
                    nc.vector.scalar_tensor_tensor(
                        out=macc, in0=w5, scalar=rp, in1=cmx(f"wv{e}"),
                        op0=AL.mult, op1=AL.mult)
                    maccs.append(macc)
                    w7 = workp.tile([C, D], F32, tag="w")
                    bacc = perp.tile([C, 1], F32, tag=f"bacc{e}")
                    nc.vector.scalar_tensor_tensor(
                        out=w7, in0=w5, scalar=rp, in1=cmx(f"uw{e}"),
                        op0=AL.mult, op1=AL.mult, accum_out=bacc)
                    d[f"bacc{e}"] = bacc
                mcast = perp.tile([C, D], FP16, tag="mcast")
                nc.vector.tensor_tensor(out=mcast, in0=maccs[0], in1=maccs[1],
                                        op=AL.add)
                tmpb = smallp.tile([C, 1], F32, tag="sm")
                nc.vector.tensor_tensor(out=tmpb, in0=d["bacc0"],
                                        in1=d["bacc1"], op=AL.add)
                beta = perp.tile([C, 1], F32, tag="beta")
                nc.vector.tensor_tensor(out=beta, in0=tmpb, in1=ccx("bfus"),
                                        op=AL.add)
                d["beta4"] = beta
                # M^T of each stripe's (identical) M into diag position s.
                # Operands stay at partition 0; only out APs are offset.
                mt_ps = auxp.tile([C, 512], F32, tag="aux")
                for s in range(NST):
                    nc.tensor.matmul(mt_ps[s * D:(s + 1) * D,
                                           s * D:(s + 1) * D],
                                     lhsT=mcast[0:D, 0:D],
                                     rhs=ident[0:D, 0:D],
                                     start=True, stop=True,
                                     tile_position=(0, s * D))
                    nc.vector.tensor_copy(m4[s * D:(s + 1) * D,
                                             s * D:(s + 1) * D],
                                          mt_ps[s * D:(s + 1) * D,
                                                s * D:(s + 1) * D])

            # ---------------- phase 2 ----------------------------------
            def emit_p2(jcs):
                for jc in jcs:
                    o_sb = osbp.tile([C, CH], F32, tag="osb")
                    for k in range(CH // 512):
                        ps = p2p.tile([C, 512], F32, tag="ps")
                        nc.tensor.matmul(
                            ps, lhsT=m4,
                            rhs=xres[jc][:, k * 512:(k + 1) * 512],
                            start=True, stop=True)
                        dst = o_sb[:, k * 512:(k + 1) * 512]
                        r = state["cp_rot"] % 2
                        state["cp_rot"] += 1
                        if r == 0:
                            nc.vector.tensor_scalar_add(dst, in0=ps,
                                                        scalar1=d["beta4"])
                        else:
                            nc.scalar.add(dst, ps, d["beta4"])
                    dst_d = out[:, jc * CH:(jc + 1) * CH]
                    if state["st_tog"] % 2 == 0:
                        nc.scalar.dma_start(out=dst_d, in_=o_sb)
                    else:
                        nc.sync.dma_start(out=dst_d, in_=o_sb)
                    state["st_tog"] += 1

            # ================= emission schedule =======================
            emit_tg_groups(range(SW // CH))     # 8 chunks
            emit_grams(pend[0])                 # last group
            emit_fold()
            emit_mbuild()
            emit_p2(range(SW // CH))

    nc.compile()
    return nc


def _host_consts(core, w_qkv, b_qkv, w_fus, b_fus, t):
    """Per-core consts for (batch b, block n) = divmod(core, NCH)."""
    _, n = divmod(core, NCH)
    sl = slice(n * D, (n + 1) * D)
    t = t.reshape(E * NCH)

    cmats = np.zeros((D, NCM, D), np.float64)
    ccols = np.zeros((D, NCC), np.float64)
    cmats[:, CMAT["ident"], :] = np.eye(D)
    ccols[:, CCOL["bfus"]] = b_fus[sl]

    for e in range(E):
        A = w_qkv[sl, e]; Bv = b_qkv[sl, e]
        Cv = w_qkv[sl, E + e]; Dv = b_qkv[sl, E + e]
        Vv = w_qkv[sl, 2 * E + e]; Uv = b_qkv[sl, 2 * E + e]
        wf = w_fus[sl, e]
        tau = t[e * NCH + n]
        cmats[:, CMAT[f"crow{e}"], :] = np.broadcast_to(Cv[None, :], (D, D))
        cmats[:, CMAT[f"drow{e}"], :] = np.broadcast_to(Dv[None, :], (D, D))
        cmats[:, CMAT[f"wv{e}"], :] = wf[:, None] * Vv[None, :]
        cmats[:, CMAT[f"uw{e}"], :] = wf[:, None] * Uv[None, :]
        ccols[:, CCOL[f"c{e}"]] = Cv
        ccols[:, CCOL[f"nd{e}"]] = NFULL * Dv
        ccols[:, CCOL[f"ta{e}"]] = tau * A
        ccols[:, CCOL[f"tb{e}"]] = tau * Bv
        ccols[:, CCOL[f"a2{e}"]] = A * A
        ccols[:, CCOL[f"ab2{e}"]] = 2 * A * Bv
        ccols[:, CCOL[f"b2n{e}"]] = NFULL * Bv * Bv
        ccols[:, CCOL[f"c2{e}"]] = Cv * Cv
        ccols[:, CCOL[f"cd2{e}"]] = 2 * Cv * Dv
        ccols[:, CCOL[f"d2n{e}"]] = NFULL * Dv * Dv

    # replicate across the 4 stripe partition groups -> [128, ...]
    cmats = np.tile(cmats, (NST, 1, 1))
    ccols = np.tile(ccols, (NST, 1))
    return cmats.astype(np.float32), ccols.astype(np.float32)


def kernel(x, w_qkv, b_qkv, w_fus, b_fus, t, _profile=None):
    x = np.asarray(x, dtype=np.float32)
    w_qkv = np.asarray(w_qkv, dtype=np.float64)
    b_qkv = np.asarray(b_qkv, dtype=np.float64)
    w_fus = np.asarray(w_fus, dtype=np.float64)
    b_fus = np.asarray(b_fus, dtype=np.float64)
    t = np.asarray(t, dtype=np.float64)

    if "hs" not in _cache:
        _cache["hs"] = _build()
    nc = _cache["hs"]

    idd = np.eye(C, dtype=np.float16)
    # fold+replicate selector: out[32s'+j, i] = sum_s gp[32s+j, i]
    st4 = np.tile(np.eye(D, dtype=np.float32), (NST, NST))     # [128, 128]
    # stack rows: [cs0, cs1, rnk0, rnk1]; sel_cs_e = one-hot row e,
    # sel_rnk_e = one-hot row 2+e (each [4,128], broadcast to all partitions)
    selm = np.zeros((NST, 4 * C), np.float16)
    for e in range(E):
        selm[e, 256 * e:256 * e + 128] = 1.0
        selm[2 + e, 256 * e + 128:256 * e + 256] = 1.0

    xf = x.reshape(B, C, HW)
    in_maps = []
    for core in range(NCORES):
        b, n = divmod(core, NCH)
        # [32, HW] -> stripes [4, 32, SW] -> [128, SW]
        xs = np.ascontiguousarray(
            xf[b, n * D:(n + 1) * D].reshape(D, NST, SW)
            .transpose(1, 0, 2).reshape(C, SW))
        cmats, ccols = _host_consts(core, w_qkv, b_qkv, w_fus, b_fus, t)
        in_maps.append({"x": xs, "idd": idd, "st4": st4,
                        "cmats": cmats, "ccols": ccols, "selm": selm})

    kw = {}
    if _profile and _profile.get("trace"):
        kw["trace"] = True
    res = bass_utils.run_bass_kernel_spmd(
        nc, in_maps, core_ids=list(range(NCORES)), **kw)
    out = np.empty((B, C, HW), np.float32)
    for core in range(NCORES):
        b, n = divmod(core, NCH)
        o = res.results[core]["out"].reshape(NST, D, SW)
        out[b, n * D:(n + 1) * D] = o.transpose(1, 0, 2).reshape(D, HW)
    if _profile is not None:
        _profile["results"] = res
    return out.reshape(B, C, H, W)
